# revision 16
# baseline (speedup 1.0000x reference)
"""Trainium2 Bass kernel for nn_MARMeansCovsLayer.

Reference computation (B=4, T=8192, C=16, L=5):
  mu_t[b,t,i] = sum_{l=0}^{4} sum_j coeffs_lt[b,(t+DL[l])%T, l, i, j] * data[b,(t-l)%T, j]
  with DL = [0, 0, 1, 3, 6]  (DL[l] = l*(l-1)/2, from the cumulative rolls)
  x_t     = data[:, 6:8192]          (= roll(data,-1)[:, 5:-1])
  mu_t    = mu[:, 5:8191]
  sigma_t = covs_t[:, 5:8191]

Sharding: 8 cores = 4 batches x 2 time-halves. The host bakes the per-lag
time shifts into one per-core slab whose row r holds the 5 lag-shifted 16x16
coeff matrices (1280 f32) followed by the 5 lag-shifted data rows (80 f32).
The device computes mu: per 128-row group, one broadcast multiply (i-major
product layout) + one segmented X-reduce on the vector engine. x_t/sigma_t
are pure input slices and are assembled on the host.

This walrus build fits very few sync waits per instruction (TT/DMA: 1), and
the kernel-tail drain waits once per live DMA sem lane - so the kernel uses
exactly TWO DMAs (one slab load, one mu store) to keep the drain small.
"""

import os
import sys

import numpy as np

B, T, C, L = 4, 8192, 16, 5
TOUT = T - L - 1          # 8186
TC = TOUT // 2            # 4093 output rows per core
DL = (0, 0, 1, 3, 6)      # coeff time offset per lag
P = 128                   # partition rows per SBUF group
G = 32                    # groups (G*P = 4096 >= TC)
CC = C * C                # 256
CSW = L * C * C           # 1280
DSW = L * C               # 80
ROW = CSW + DSW           # 1360 floats per slab row

_NC = None                # cached Bass program
LAST_EXEC_NS = None       # HW exec time of last run (when tracing)
LAST_RESULT = None


def _import_bass():
    try:
        import concourse.bass  # noqa: F401
    except ImportError:
        sys.path.insert(0, "/opt/trn_rl_repo")


def _build_program():
    import concourse.bacc as bacc
    import concourse.tile as tile
    from concourse import mybir

    nc = bacc.Bacc(None)
    dt = mybir.dt.float32

    slab = nc.dram_tensor("slab", [G * P, ROW], dt, kind="ExternalInput")
    mu = nc.dram_tensor("mu", [G * P, C], dt, kind="ExternalOutput")

    with tile.TileContext(nc) as tc:
        with (
            tc.tile_pool(name="slab_pool", bufs=1) as slab_pool,
            tc.tile_pool(name="prod_pool", bufs=2) as prod_pool,
            tc.tile_pool(name="mu_pool", bufs=1) as mu_pool,
        ):
            slab_t = slab_pool.tile([P, G * ROW], dt, tag="slab")
            nc.gpsimd.dma_start(
                out=slab_t[:, :].rearrange("p (g c) -> p g c", g=G),
                in_=slab[:, :].rearrange("(g p) c -> p g c", g=G, p=P),
            )
            mu_t = mu_pool.tile([P, G * C], dt, tag="mu")
            for g in range(G):
                base = g * ROW
                cs_v = slab_t[:, base:base + CSW].rearrange(
                    "p (l i j) -> p l i j", l=L, i=C, j=C
                )
                d4 = (
                    slab_t[:, base + CSW:base + ROW]
                    .rearrange("p (l j) -> p l j", l=L)
                    .unsqueeze(2)
                    .broadcast_to([P, L, C, C])
                )
                prod_t = prod_pool.tile([P, CSW], dt, tag="prod")
                # write product i-major so one X-reduce sums over (l, j)
                prod_v = prod_t[:, :].rearrange(
                    "p (i l j) -> p l i j", i=C, l=L, j=C
                )
                nc.vector.tensor_tensor(
                    out=prod_v, in0=cs_v, in1=d4, op=mybir.AluOpType.mult
                )
                nc.vector.tensor_reduce(
                    out=mu_t[:, g * C:(g + 1) * C],
                    in_=prod_t[:, :].rearrange("p (i lj) -> p i lj", i=C),
                    axis=mybir.AxisListType.X,
                    op=mybir.AluOpType.add,
                )
            nc.gpsimd.dma_start(
                out=mu[:, :].rearrange("(g p) c -> p g c", g=G, p=P),
                in_=mu_t[:, :].rearrange("p (g c) -> p g c", g=G),
            )
    nc.finalize()
    return nc


def _get_nc():
    global _NC
    if _NC is None:
        _import_bass()
        _NC = _build_program()
    return _NC


def kernel(data, coeffs_lt, covs_t, n_lags):
    global LAST_EXEC_NS, LAST_RESULT
    data = np.ascontiguousarray(np.asarray(data, dtype=np.float32))
    coeffs_lt = np.ascontiguousarray(np.asarray(coeffs_lt, dtype=np.float32))
    covs_t = np.ascontiguousarray(np.asarray(covs_t, dtype=np.float32))
    assert int(n_lags) == L, f"kernel hardcodes n_lags={L}, got {n_lags}"
    assert data.shape == (B, T, C)
    assert coeffs_lt.shape == (B, T, L, C, C)
    assert covs_t.shape == (B, T, C, C)

    nc = _get_nc()
    from concourse.bass_utils import run_bass_kernel_spmd

    r = np.arange(TC)
    in_maps = []
    for core in range(8):
        b, half = divmod(core, 2)
        t0 = 5 + half * TC
        slab_m = np.zeros((G * P, ROW), dtype=np.float32)
        for l in range(L):
            slab_m[:TC, l * CC:(l + 1) * CC] = (
                coeffs_lt[b][(t0 + r + DL[l]) % T, l].reshape(TC, CC)
            )
            slab_m[:TC, CSW + l * C:CSW + (l + 1) * C] = (
                data[b][(t0 + r - l) % T]
            )
        in_maps.append({"slab": slab_m})

    trace = bool(int(os.environ.get("BASS_KERNEL_TRACE", "0")))
    try:
        res = run_bass_kernel_spmd(
            nc, in_maps, core_ids=list(range(8)), trace=trace
        )
    except ModuleNotFoundError:
        # axon NTFF profiling hook unavailable in this container
        res = run_bass_kernel_spmd(nc, in_maps, core_ids=list(range(8)))
    LAST_RESULT = res
    LAST_EXEC_NS = getattr(res, "exec_time_ns", None)

    x_t = np.empty((B, TOUT, C), dtype=np.float32)
    mu_t = np.empty((B, TOUT, C), dtype=np.float32)
    sigma_t = np.empty((B, TOUT, C, C), dtype=np.float32)
    for core in range(8):
        b, half = divmod(core, 2)
        sl = slice(half * TC, (half + 1) * TC)
        t0 = 5 + half * TC
        mu_t[b, sl] = res.results[core]["mu"][:TC]
        x_t[b, sl] = data[b, t0 + 1:t0 + TC + 1]
        sigma_t[b, sl] = covs_t[b, t0:t0 + TC]
    return x_t, mu_t, sigma_t
